# revision 47
# baseline (speedup 1.0000x reference)
"""Fast-weight-sum (causal linear attention) transformer layer on 8 TRN2 cores.

Sharding: data-parallel over batch - BSZ=8 batches, one per NeuronCore, no
collectives. Each core runs the full layer for its batch column of h.

v2 vs baseline:
  * qkv projection in fp8e4 with DoubleRow perf mode (2 k-tiles per matmul,
    0.5 cycles/row) - the single largest PE cost drops ~4x.
  * Fused scalar_tensor_tensor ops: feature map relu+add in one DVE op,
    v_ext scale (SCALE * krec) in one DVE op.
  * EPS folded into the attention state's k_state column (seeded by a tiny
    rank-1 matmul), so the denominator is a single PSUM column: the per-chunk
    ACT "st" copies disappear; the epilogue reads PSUM directly on DVE.
  * Masks applied one DVE op per 4 heads (scores for 4 heads share a bank).
  * min(exp, 1) runs on the otherwise-idle GpSimd engine; ksum reduce too.
  * Issue order is software-pipelined (qkv(c+2) / attn(c) / outproj(c-1))
    to keep the PE stream dense so it ramps to the 2.4 GHz p-state.

Per-core algorithm (L=1024, D=512, H=8 heads, dh=64, chunk C=128):
  qkv = h @ W_qkv (fp8 DoubleRow; W pre-permuted to [Q|K|V] blocks on host)
  q <- elu(q)+1          (NOT normalized; 1/sum_d(q) cancels num/denom)
  k <- elu(k)+1 unnormalized; 1/sum_d(k) folded into v_ext per position
  Chunked causal linear attention, running state S_h = sum khat vhat^T plus a
  k_state column (+EPS seed). Per chunk c, pair p = heads (2p, 2p+1):
    A^T[s,t] = k_s . q_t       (PE; 4 heads share a PSUM bank)
    am       = A^T * uppertri  (one DVE op per bank)
    pb[t, 0:65]   = am_A^T @ [SCALE*krec*v_A | krec] + q @ [S_A | kst_A+EPS]
    pb[t, 65:130] = same for head B
    sps      += k_pair^T @ v_ext_pair      (PSUM accumulation)
  attn[t] = pb[:, 0:64] / pb[:, 64]  (den col; DVE from PSUM)
  layer_out = attn @ W_o (bf16); out = layernorm(h + layer_out)
"""

import numpy as np

import concourse.bass as bass
import concourse.mybir as mybir
import concourse.tile as tile
from concourse import bacc
from concourse.bass_utils import run_bass_kernel_spmd


L, D, F, H, DH = 1024, 512, 1536, 8, 64
P = 128
NLT = L // P          # 8 l-tiles == chunks
KD = D // P           # 4 contraction tiles of d_model
EPS = 1e-5
LN_EPS = 1e-5
SCALE = 1.0 / np.sqrt(DH)
BF = mybir.dt.bfloat16
F32 = mybir.dt.float32
FP8 = mybir.dt.float8e4
AX = mybir.AluOpType
ACTF = mybir.ActivationFunctionType
DR = mybir.MatmulPerfMode.DoubleRow
QKV_MODE = "dr"  # dr | fp8 | bf16
PIPELINED = True
DEBUG_ATTN = False
USE_STT = True
BATCH_MASKS = True
SEXT_SPLIT = False
MASKS_ON_GPSIMD = False
USE_SEEDS = False
QKV_DT = None  # set below

SE = DH + 1           # 65: per-head state cols [S(64) | k_state]
PBW = 2 * SE          # 130: per-pair pb cols

LAST_RESULT = None


def _build_core_kernel(nc, tc, apply_gb=True):
    # fp8 operands for the qkv projection; bf16 h for the residual path.
    h_d = nc.dram_tensor("h", (P, NLT, D), BF, kind="ExternalInput")
    qdt = BF if QKV_MODE == "bf16" else FP8
    ident_d = nc.dram_tensor("ident", (P, P), BF, kind="ExternalInput")
    hT8_d = nc.dram_tensor("hT8", (P, NLT, KD, P), qdt, kind="ExternalInput")
    wq8_d = nc.dram_tensor("W_qkv8", (P, KD, F), qdt, kind="ExternalInput")
    wo_d = nc.dram_tensor("W_o", (P, KD, D), BF, kind="ExternalInput")
    gamma_d = nc.dram_tensor("gamma", (D,), F32, kind="ExternalInput")
    beta_d = nc.dram_tensor("beta", (D,), F32, kind="ExternalInput")
    out_d = nc.dram_tensor("out", (L, D), F32, kind="ExternalOutput")
    attn_dbg = nc.dram_tensor("attn_dbg", (P, NLT, D), BF,
                              kind="ExternalOutput") if DEBUG_ATTN else None

    with (
        tc.tile_pool(name="consts", bufs=1) as consts,
        tc.tile_pool(name="work", bufs=3) as work,
        tc.tile_pool(name="pmm", bufs=2, space="PSUM") as pmm,
        tc.tile_pool(name="pab", bufs=3, space="PSUM") as pab,
        tc.tile_pool(name="pstate", bufs=1, space="PSUM") as pstate,
    ):
        # ---------- input DMAs ----------
        wq8 = consts.tile([P, KD, F], qdt, tag="wq8")
        hT8 = consts.tile([P, NLT, KD, P], qdt, tag="hT8")
        h_bf = consts.tile([P, NLT, D], BF, tag="h_bf")
        wo_b = consts.tile([P, KD, D], BF, tag="wo_b")

        ident_sb = consts.tile([P, P], BF, tag="ident_sb")
        nc.gpsimd.dma_start(ident_sb, ident_d[:])
        nc.sync.dma_start(wq8, wq8_d[:])
        nc.scalar.dma_start(hT8, hT8_d[:])
        nc.scalar.dma_start(h_bf, h_d[:])
        nc.gpsimd.dma_start(wo_b, wo_d[:])

        # ---------- constants ----------
        # Causal mask replicated for 4 heads: utri4[s, j, t] = 1 iff s <= t.
        utri4 = consts.tile([P, 4, P], BF, tag="utri4")
        nc.gpsimd.memset(utri4, 0.0)
        nc.gpsimd.affine_select(
            out=utri4, in_=utri4, compare_op=AX.is_gt, fill=1.0,
            base=0, pattern=[[0, 4], [-1, P]], channel_multiplier=1,
        )

        gamma_ap = gamma_d[:]
        gamma_bc = consts.tile([P, D], BF, tag="gamma_bc")
        nc.gpsimd.dma_start(
            gamma_bc,
            bass.AP(tensor=gamma_ap.tensor, offset=gamma_ap.offset,
                    ap=[[0, P]] + list(gamma_ap.ap)),
        )
        beta_ap = beta_d[:]
        beta_bc = consts.tile([P, D], F32, tag="beta_bc")
        nc.gpsimd.dma_start(
            beta_bc,
            bass.AP(tensor=beta_ap.tensor, offset=beta_ap.offset,
                    ap=[[0, P]] + list(beta_ap.ap)),
        )
        eps_sb = consts.tile([P, 1], F32, tag="eps_sb")
        nc.vector.memset(eps_sb, LN_EPS)
        # EPS seed operands for the k_state column (rank-1 matmul):
        # ones_row [1, 128] as lhsT, eps_row [1, 2] as rhs.
        # K=128 operands for the PSUM zero/seed matmuls (K<128 matmuls are
        # flaky on hardware): ones128^T @ eps_col broadcasts EPS/128 * 128.
        ones128 = consts.tile([P, P], BF, tag="ones128")
        nc.gpsimd.memset(ones128, 1.0)
        eps_col = consts.tile([P, 2], F32, tag="eps_col")
        nc.vector.memset(eps_col, EPS / P)
        ecb = consts.tile([P, 2], BF, tag="ecb")
        nc.vector.tensor_copy(ecb, eps_col)
        zero_row = consts.tile([P, 2 * PBW], BF, tag="zero_row")
        nc.gpsimd.memset(zero_row, 0.0)

        # q and k share one tensor: qk_sb[:, lt, 0:512] = q, [:, 512:1024] = k.
        qk_sb = consts.tile([P, NLT, 2 * D], BF, tag="qk_sb")
        qkT = consts.tile([P, NLT, 8, P], BF, tag="qkT")
        # v_ext[s, lt, h, 0:64] = SCALE * krec[s,h] * v_h[s,:]; col 64 = krec.
        v_ext = consts.tile([P, NLT, H, SE], BF, tag="v_ext")
        # Per-pair state in SBUF for the inter matmul rhs:
        # s_ext[:, p, :] = [S_A(64) | kstA+EPS] rows 0:64 / [S_B | kstB+EPS]
        # rows 64:128 block-diagonal (off-blocks stay zero).
        s_ext2 = consts.tile([P, 2, 4, PBW], BF, tag="s_ext2")
        nc.gpsimd.memset(s_ext2, 0.0)
        attn = consts.tile([P, NLT, D], BF, tag="attn")
        attnT = consts.tile([P, NLT, KD, P], BF, tag="attnT")
        x_all = consts.tile([P, NLT, D], BF, tag="x_all")
        mv_all = consts.tile([P, NLT, 2], F32, tag="mv_all")

        # Per-(parity, pair) S accumulators: 8 units of [128, 130] packed
        # 3-per-bank into 3 PSUM banks. Unit k = parity*4 + pair.
        s_ps = [pstate.tile([P, 3, PBW], F32, tag=f"sp{i}", name=f"s_ps{i}")
                for i in range(3)]

        def s_unit(par, p):
            k = par * 4 + p
            return s_ps[k // 3][:, k % 3, :]

        def qkv_phase(lt):
            for g in range(3):  # 0=q, 1=k, 2=v
                pm = pmm.tile([P, D], F32, tag="mm")
                if QKV_MODE == "dr":
                    for j in range(KD // 2):
                        nc.tensor.matmul(
                            pm,
                            lhsT=hT8[:, lt, 2 * j:2 * j + 2, :],
                            rhs=wq8[:, 2 * j:2 * j + 2, g * D:(g + 1) * D],
                            start=(j == 0),
                            stop=(j == KD // 2 - 1),
                            perf_mode=DR,
                        )
                else:
                    for kt in range(KD):
                        nc.tensor.matmul(
                            pm,
                            lhsT=hT8[:, lt, kt, :],
                            rhs=wq8[:, kt, g * D:(g + 1) * D],
                            start=(kt == 0),
                            stop=(kt == KD - 1),
                        )
                if g == 2:
                    # v_ext <- pm * (SCALE * krec) (broadcast per head)
                    kr = krec.rearrange("p (h o) -> p h o", o=1)
                    krs = work.tile([P, H], F32, tag="krs")
                    nc.vector.tensor_scalar_mul(krs, krec, SCALE)
                    nc.vector.tensor_tensor(
                        v_ext[:, lt, :, 0:DH],
                        pm.rearrange("p (h e) -> p h e", e=DH),
                        krs.rearrange("p (h o) -> p h o", o=1).to_broadcast(
                            (P, H, DH)),
                        AX.mult,
                    )
                    nc.scalar.copy(v_ext[:, lt, :, DH:SE], kr)
                else:
                    # elu(x)+1 == relu(x) + min(exp(x), 1)
                    dst = qk_sb[:, lt, g * D:(g + 1) * D]
                    if USE_STT:
                        e1 = work.tile([P, D], BF, tag="fmap_e")
                        nc.scalar.activation(e1, pm, ACTF.Exp)
                        em = work.tile([P, D], BF, tag="fmap_m")
                        nc.vector.tensor_scalar_min(em, e1, 1.0)
                        nc.vector.scalar_tensor_tensor(
                            out=dst, in0=pm, scalar=0.0, in1=em,
                            op0=AX.max, op1=AX.add,
                        )
                    else:
                        e1 = work.tile([P, D], BF, tag="fmap_e")
                        nc.scalar.activation(e1, pm, ACTF.Exp)
                        nc.vector.tensor_scalar_min(e1, e1, 1.0)
                        r1 = work.tile([P, D], BF, tag="fmap_r")
                        nc.scalar.activation(r1, pm, ACTF.Relu)
                        nc.vector.tensor_add(out=dst, in0=e1, in1=r1)
                    if g == 1:
                        ksum = work.tile([P, H], F32, tag="ksum")
                        nc.vector.reduce_sum(
                            out=ksum,
                            in_=dst.rearrange("p (h e) -> p h e", e=DH),
                            axis=mybir.AxisListType.X,
                        )
                        krec = work.tile([P, H], F32, tag="krec")
                        nc.vector.reciprocal(krec, ksum)
            nc.scalar.dma_start_transpose(qkT[:, lt], qk_sb[:, lt])

        def attn_scores(c):
            # scores for the 8 heads, parity-grouped per PSUM bank (mixing
            # PE base partitions within one bank back-to-back crashes the
            # exec unit), then inter + S-update. pb gets its own ring tiles
            # so inter does NOT wait for the masks (which read the scores
            # banks).
            ams = []
            abpb = []
            for par in range(2):  # bank holds heads {par, par+2, ...}
                ab = pab.tile([P, 512], F32, tag="ab", name="ab")
                abpb.append(ab)
                ho = par * DH
                for hh in range(4):
                    h = 2 * hh + par
                    nc.tensor.matmul(
                        ab[:, hh * P:(hh + 1) * P],
                        lhsT=qkT[ho:ho + DH, c, 4 + h // 2, :],
                        rhs=qkT[ho:ho + DH, c, h // 2, :],
                        start=True, stop=True,
                    )
                am = work.tile([P, 4, P], BF, tag="am", name="am")
                nc.vector.tensor_tensor(
                    am, ab.rearrange("p (j t) -> p j t", j=4), utri4,
                    AX.mult)
                ams.append(am)
            pbt_a = pab.tile([P, 512], F32, tag="ab", name="pbA")
            pbt_b = pab.tile([P, 512], F32, tag="ab", name="pbB")
            pba = pbt_a[:, 0:3 * PBW].rearrange("p (t e) -> p t e", e=PBW)
            pbb = pbt_b[:, 0:PBW].rearrange("p (t e) -> p t e", e=PBW)

            def pb_of(p):
                return pba[:, p, :] if p < 3 else pbb[:, 0, :]

            return ams, pba, pbb, pb_of

        def attn_pairs(c, ams, pba, pbb, pb_of):
            # Pair-contiguous inter -> intra -> S: a start=True into a bank
            # invalidates other regions' pending accumulation, so each
            # pair's group must fully finish before the next pair opens.
            for p in range(4):
                pb = pb_of(p)
                first = True
                for par in range(2):
                    if c <= par:
                        continue  # that accumulator is still empty
                    nc.tensor.matmul(pb, lhsT=qkT[:, c, p, :],
                                     rhs=s_ext2[:, par, p, :],
                                     start=first, stop=False,
                                     skip_group_check=True)
                    first = False
                for j in range(2):
                    h = 2 * p + j
                    amh = ams[h % 2][:, h // 2, :]
                    nc.tensor.matmul(
                        pb[:, j * SE:(j + 1) * SE],
                        lhsT=amh,
                        rhs=v_ext[:, c, h, :],
                        start=first, stop=(j == 1),
                        skip_group_check=True)
                # S update into this chunk's parity accumulator. EPS is
                # dropped: den is strictly positive. stop=True each chunk
                # is a hardware no-op but closes the sim accumulation group
                # so the s_ext copies may read the state.
                par = c % 2
                sps = s_unit(par, p)
                if c < NLT - 1:
                    nc.tensor.matmul(
                        sps,
                        lhsT=qk_sb[:, c,
                                   D + 2 * p * DH:D + (2 * p + 2) * DH],
                        rhs=v_ext[:, c, 2 * p:2 * p + 2, :],
                        start=(c == par), stop=True,
                        skip_group_check=True)
                    # refresh this parity's SBUF snapshot; next read is at
                    # chunk c+2, so these copies are off the critical path.
                    nc.scalar.copy(s_ext2[0:DH, par, p, 0:SE],
                                   sps[0:DH, 0:SE])
                    nc.scalar.copy(s_ext2[DH:P, par, p, SE:PBW],
                                   sps[DH:P, SE:PBW])


        def attn_epi(c, ams, pba, pbb, pb_of):
            # epilogue: strided den cols -> reciprocal -> scale out cols
            den = work.tile([P, H], F32, tag="den")
            nc.scalar.copy(
                den[:, 0:6].rearrange("p (f o) -> p f o", o=1),
                pba.rearrange("p t (j e) -> p (t j) e", e=SE)[:, :, DH:SE])
            nc.scalar.copy(
                den[:, 6:8].rearrange("p (f o) -> p f o", o=1),
                pbb.rearrange("p t (j e) -> p (t j) e", e=SE)[:, :, DH:SE])
            denr = work.tile([P, H], F32, tag="denr")
            nc.vector.reciprocal(denr, den)
            dr8 = denr.rearrange("p (j o) -> p j o", o=1)
            nc.vector.tensor_tensor(
                attn[:, c, 0:6 * DH].rearrange("p (j e) -> p j e", e=DH),
                pba.rearrange("p t (j e) -> p (t j) e", e=SE)[:, :, 0:DH],
                dr8[:, 0:6].to_broadcast((P, 6, DH)),
                AX.mult,
            )
            nc.vector.tensor_tensor(
                attn[:, c, 6 * DH:D].rearrange("p (j e) -> p j e", e=DH),
                pbb.rearrange("p t (j e) -> p (t j) e", e=SE)[:, :, 0:DH],
                dr8[:, 6:8].to_broadcast((P, 2, DH)),
                AX.mult,
            )
            nc.sync.dma_start_transpose(attnT[:, c], attn[:, c])
            if DEBUG_ATTN:
                nc.sync.dma_start(attn_dbg[:, c], attn[:, c])

        def outproj_phase(lt):
            # Matmuls + stats only; every mid-stream ACT op here is a Copy,
            # which lives in the same act table as the feature map's Exp -
            # no act-table reloads until the final ln_phase.
            pm = pmm.tile([P, D], F32, tag="mm")
            for kt in range(KD):
                nc.tensor.matmul(pm, lhsT=attnT[:, lt, kt, :],
                                 rhs=wo_b[:, kt], start=(kt == 0),
                                 stop=False)
            # residual add on the PE: pm += I^T @ h
            nc.tensor.matmul(pm, lhsT=ident_sb, rhs=h_bf[:, lt],
                             start=False, stop=True)
            nc.scalar.copy(x_all[:, lt], pm)
            stats = work.tile([P, nc.vector.BN_STATS_DIM], F32, tag="stats")
            nc.vector.bn_stats(out=stats, in_=x_all[:, lt])
            nc.vector.bn_aggr(out=mv_all[:, lt], in_=stats)

        def ln_phase(lt):
            mv = mv_all[:, lt]
            std = work.tile([P, 1], F32, tag="std")
            nc.scalar.activation(std, mv[:, 1:2], ACTF.Sqrt, bias=eps_sb,
                                 scale=1.0)
            rstd = work.tile([P, 1], F32, tag="rstd")
            nc.vector.reciprocal(rstd, std)
            nmr = work.tile([P, 1], F32, tag="nmr")
            nc.vector.tensor_scalar(out=nmr, in0=mv[:, 0:1], scalar1=-1.0,
                                    scalar2=rstd, op0=AX.mult, op1=AX.mult)
            xn = work.tile([P, D], F32 if not apply_gb else BF, tag="xn")
            nc.scalar.activation(xn, x_all[:, lt], ACTF.Identity, bias=nmr,
                                 scale=rstd)
            if apply_gb:
                xg = work.tile([P, D], BF, tag="xg")
                nc.vector.tensor_tensor(xg, xn, gamma_bc, AX.mult)
                yo = work.tile([P, D], F32, tag="yo")
                nc.vector.tensor_tensor(yo, xg, beta_bc, AX.add)
                nc.scalar.dma_start(out_d[lt * P:(lt + 1) * P, :], yo)
            else:
                nc.scalar.dma_start(out_d[lt * P:(lt + 1) * P, :], xn)

        # ---------- issue order ----------
        qkv_phase(0)
        qkv_phase(1)
        for c in range(NLT):
            st = attn_scores(c)
            if c + 2 < NLT:
                qkv_phase(c + 2)
            if c >= 1:
                outproj_phase(c - 1)
            attn_pairs(c, *st)
            attn_epi(c, *st)
        outproj_phase(NLT - 1)
        for lt in range(NLT):
            ln_phase(lt)


_NC_CACHE = {}


def _get_nc(apply_gb=True):
    key = ("nc", apply_gb)
    if key not in _NC_CACHE:
        nc = bacc.Bacc("TRN2", target_bir_lowering=False, debug=False)
        with tile.TileContext(nc) as tc:
            _build_core_kernel(nc, tc, apply_gb=apply_gb)
        nc.compile()
        _NC_CACHE[key] = nc
    return _NC_CACHE[key]


def kernel(h, W_qkv, W_o, gamma, beta, trace=False):
    global LAST_RESULT
    h = np.asarray(h, dtype=np.float32)
    W_qkv = np.asarray(W_qkv, dtype=np.float32)
    W_o = np.asarray(W_o, dtype=np.float32)
    gamma = np.asarray(gamma, dtype=np.float32)
    beta = np.asarray(beta, dtype=np.float32)

    import ml_dtypes
    bf16 = ml_dtypes.bfloat16
    fp8 = ml_dtypes.float8_e4m3 if QKV_MODE != "bf16" else ml_dtypes.bfloat16
    # Permute W_qkv columns from per-head [q|k|v] interleave to [Q|K|V]
    # blocks, partition-major fp8.
    w_perm = np.ascontiguousarray(
        W_qkv.reshape(D, H, 3, DH).transpose(0, 2, 1, 3).reshape(D, F)
        .reshape(KD, P, F).transpose(1, 0, 2)).astype(fp8)
    wo_shuf = np.ascontiguousarray(
        W_o.reshape(KD, P, D).transpose(1, 0, 2)).astype(bf16)

    apply_gb = not (np.all(gamma == 1.0) and np.all(beta == 0.0))
    nc = _get_nc(apply_gb)
    in_maps = []
    for b in range(8):
        hb = h[:, b, :]
        in_maps.append({
            "ident": np.eye(P, dtype=np.float32).astype(bf16),
            "h": np.ascontiguousarray(
                hb.reshape(NLT, P, D).transpose(1, 0, 2)).astype(bf16),
            "hT8": np.ascontiguousarray(
                hb.reshape(NLT, P, KD, P).transpose(3, 0, 2, 1)).astype(fp8),
            "W_qkv8": w_perm,
            "W_o": wo_shuf,
            "gamma": gamma,
            "beta": beta,
        })
    res = run_bass_kernel_spmd(nc, in_maps, core_ids=list(range(8)), trace=trace)
    LAST_RESULT = res
    return np.stack([res.results[b]["out"] for b in range(8)], axis=1)


# revision 48
# speedup vs baseline: 1.0071x; 1.0071x over previous
"""Fast-weight-sum (causal linear attention) transformer layer on 8 TRN2 cores.

Sharding: data-parallel over batch - BSZ=8 batches, one per NeuronCore, no
collectives. Each core runs the full layer for its batch column of h.

v2 vs baseline:
  * qkv projection in fp8e4 with DoubleRow perf mode (2 k-tiles per matmul,
    0.5 cycles/row) - the single largest PE cost drops ~4x.
  * Fused scalar_tensor_tensor ops: feature map relu+add in one DVE op,
    v_ext scale (SCALE * krec) in one DVE op.
  * EPS folded into the attention state's k_state column (seeded by a tiny
    rank-1 matmul), so the denominator is a single PSUM column: the per-chunk
    ACT "st" copies disappear; the epilogue reads PSUM directly on DVE.
  * Masks applied one DVE op per 4 heads (scores for 4 heads share a bank).
  * min(exp, 1) runs on the otherwise-idle GpSimd engine; ksum reduce too.
  * Issue order is software-pipelined (qkv(c+2) / attn(c) / outproj(c-1))
    to keep the PE stream dense so it ramps to the 2.4 GHz p-state.

Per-core algorithm (L=1024, D=512, H=8 heads, dh=64, chunk C=128):
  qkv = h @ W_qkv (fp8 DoubleRow; W pre-permuted to [Q|K|V] blocks on host)
  q <- elu(q)+1          (NOT normalized; 1/sum_d(q) cancels num/denom)
  k <- elu(k)+1 unnormalized; 1/sum_d(k) folded into v_ext per position
  Chunked causal linear attention, running state S_h = sum khat vhat^T plus a
  k_state column (+EPS seed). Per chunk c, pair p = heads (2p, 2p+1):
    A^T[s,t] = k_s . q_t       (PE; 4 heads share a PSUM bank)
    am       = A^T * uppertri  (one DVE op per bank)
    pb[t, 0:65]   = am_A^T @ [SCALE*krec*v_A | krec] + q @ [S_A | kst_A+EPS]
    pb[t, 65:130] = same for head B
    sps      += k_pair^T @ v_ext_pair      (PSUM accumulation)
  attn[t] = pb[:, 0:64] / pb[:, 64]  (den col; DVE from PSUM)
  layer_out = attn @ W_o (bf16); out = layernorm(h + layer_out)
"""

import numpy as np

import concourse.bass as bass
import concourse.mybir as mybir
import concourse.tile as tile
from concourse import bacc
from concourse.bass_utils import run_bass_kernel_spmd


L, D, F, H, DH = 1024, 512, 1536, 8, 64
P = 128
NLT = L // P          # 8 l-tiles == chunks
KD = D // P           # 4 contraction tiles of d_model
EPS = 1e-5
LN_EPS = 1e-5
SCALE = 1.0 / np.sqrt(DH)
BF = mybir.dt.bfloat16
F32 = mybir.dt.float32
FP8 = mybir.dt.float8e4
AX = mybir.AluOpType
ACTF = mybir.ActivationFunctionType
DR = mybir.MatmulPerfMode.DoubleRow
QKV_MODE = "dr"  # dr | fp8 | bf16
PIPELINED = True
DEBUG_ATTN = False
USE_STT = True
BATCH_MASKS = True
SEXT_SPLIT = False
MASKS_ON_GPSIMD = False
USE_SEEDS = False
QKV_DT = None  # set below

SE = DH + 1           # 65: per-head state cols [S(64) | k_state]
PBW = 2 * SE          # 130: per-pair pb cols

LAST_RESULT = None


def _build_core_kernel(nc, tc, apply_gb=True):
    # fp8 operands for the qkv projection; bf16 h for the residual path.
    h_d = nc.dram_tensor("h", (P, NLT, D), BF, kind="ExternalInput")
    qdt = BF if QKV_MODE == "bf16" else FP8
    ident_d = nc.dram_tensor("ident", (P, P), BF, kind="ExternalInput")
    hT8_d = nc.dram_tensor("hT8", (P, NLT, KD, P), qdt, kind="ExternalInput")
    wq8_d = nc.dram_tensor("W_qkv8", (P, KD, F), qdt, kind="ExternalInput")
    wo_d = nc.dram_tensor("W_o", (P, KD, D), BF, kind="ExternalInput")
    gamma_d = nc.dram_tensor("gamma", (D,), F32, kind="ExternalInput")
    beta_d = nc.dram_tensor("beta", (D,), F32, kind="ExternalInput")
    out_d = nc.dram_tensor("out", (L, D), F32, kind="ExternalOutput")
    attn_dbg = nc.dram_tensor("attn_dbg", (P, NLT, D), BF,
                              kind="ExternalOutput") if DEBUG_ATTN else None

    with (
        tc.tile_pool(name="consts", bufs=1) as consts,
        tc.tile_pool(name="work", bufs=3) as work,
        tc.tile_pool(name="pmm", bufs=2, space="PSUM") as pmm,
        tc.tile_pool(name="pab", bufs=3, space="PSUM") as pab,
        tc.tile_pool(name="pstate", bufs=1, space="PSUM") as pstate,
    ):
        # ---------- input DMAs ----------
        wq8 = consts.tile([P, KD, F], qdt, tag="wq8")
        hT8 = consts.tile([P, NLT, KD, P], qdt, tag="hT8")
        h_bf = consts.tile([P, NLT, D], BF, tag="h_bf")
        wo_b = consts.tile([P, KD, D], BF, tag="wo_b")

        ident_sb = consts.tile([P, P], BF, tag="ident_sb")
        nc.scalar.dma_start(hT8[:, 0:2], hT8_d[:, 0:2])
        nc.sync.dma_start(wq8[:, 0:2], wq8_d[:, 0:2])
        nc.scalar.dma_start(wq8[:, 2:4], wq8_d[:, 2:4])
        nc.scalar.dma_start(hT8[:, 2:NLT], hT8_d[:, 2:NLT])
        nc.gpsimd.dma_start(ident_sb, ident_d[:])
        nc.sync.dma_start(h_bf, h_d[:])
        nc.gpsimd.dma_start(wo_b, wo_d[:])

        # ---------- constants ----------
        # Causal mask replicated for 4 heads: utri4[s, j, t] = 1 iff s <= t.
        utri4 = consts.tile([P, 4, P], BF, tag="utri4")
        nc.gpsimd.memset(utri4, 0.0)
        nc.gpsimd.affine_select(
            out=utri4, in_=utri4, compare_op=AX.is_gt, fill=1.0,
            base=0, pattern=[[0, 4], [-1, P]], channel_multiplier=1,
        )

        gamma_ap = gamma_d[:]
        gamma_bc = consts.tile([P, D], BF, tag="gamma_bc")
        nc.gpsimd.dma_start(
            gamma_bc,
            bass.AP(tensor=gamma_ap.tensor, offset=gamma_ap.offset,
                    ap=[[0, P]] + list(gamma_ap.ap)),
        )
        beta_ap = beta_d[:]
        beta_bc = consts.tile([P, D], F32, tag="beta_bc")
        nc.gpsimd.dma_start(
            beta_bc,
            bass.AP(tensor=beta_ap.tensor, offset=beta_ap.offset,
                    ap=[[0, P]] + list(beta_ap.ap)),
        )
        eps_sb = consts.tile([P, 1], F32, tag="eps_sb")
        nc.vector.memset(eps_sb, LN_EPS)
        # EPS seed operands for the k_state column (rank-1 matmul):
        # ones_row [1, 128] as lhsT, eps_row [1, 2] as rhs.
        # K=128 operands for the PSUM zero/seed matmuls (K<128 matmuls are
        # flaky on hardware): ones128^T @ eps_col broadcasts EPS/128 * 128.
        ones128 = consts.tile([P, P], BF, tag="ones128")
        nc.gpsimd.memset(ones128, 1.0)
        eps_col = consts.tile([P, 2], F32, tag="eps_col")
        nc.vector.memset(eps_col, EPS / P)
        ecb = consts.tile([P, 2], BF, tag="ecb")
        nc.vector.tensor_copy(ecb, eps_col)
        zero_row = consts.tile([P, 2 * PBW], BF, tag="zero_row")
        nc.gpsimd.memset(zero_row, 0.0)

        # q and k share one tensor: qk_sb[:, lt, 0:512] = q, [:, 512:1024] = k.
        qk_sb = consts.tile([P, NLT, 2 * D], BF, tag="qk_sb")
        qkT = consts.tile([P, NLT, 8, P], BF, tag="qkT")
        # v_ext[s, lt, h, 0:64] = SCALE * krec[s,h] * v_h[s,:]; col 64 = krec.
        v_ext = consts.tile([P, NLT, H, SE], BF, tag="v_ext")
        # Per-pair state in SBUF for the inter matmul rhs:
        # s_ext[:, p, :] = [S_A(64) | kstA+EPS] rows 0:64 / [S_B | kstB+EPS]
        # rows 64:128 block-diagonal (off-blocks stay zero).
        s_ext2 = consts.tile([P, 2, 4, PBW], BF, tag="s_ext2")
        nc.gpsimd.memset(s_ext2, 0.0)
        attn = consts.tile([P, NLT, D], BF, tag="attn")
        attnT = consts.tile([P, NLT, KD, P], BF, tag="attnT")
        x_all = consts.tile([P, NLT, D], BF, tag="x_all")
        mv_all = consts.tile([P, NLT, 2], F32, tag="mv_all")

        # Per-(parity, pair) S accumulators: 8 units of [128, 130] packed
        # 3-per-bank into 3 PSUM banks. Unit k = parity*4 + pair.
        s_ps = [pstate.tile([P, 3, PBW], F32, tag=f"sp{i}", name=f"s_ps{i}")
                for i in range(3)]

        def s_unit(par, p):
            k = par * 4 + p
            return s_ps[k // 3][:, k % 3, :]

        def qkv_phase(lt):
            for g in range(3):  # 0=q, 1=k, 2=v
                pm = pmm.tile([P, D], F32, tag="mm")
                if QKV_MODE == "dr":
                    for j in range(KD // 2):
                        nc.tensor.matmul(
                            pm,
                            lhsT=hT8[:, lt, 2 * j:2 * j + 2, :],
                            rhs=wq8[:, 2 * j:2 * j + 2, g * D:(g + 1) * D],
                            start=(j == 0),
                            stop=(j == KD // 2 - 1),
                            perf_mode=DR,
                        )
                else:
                    for kt in range(KD):
                        nc.tensor.matmul(
                            pm,
                            lhsT=hT8[:, lt, kt, :],
                            rhs=wq8[:, kt, g * D:(g + 1) * D],
                            start=(kt == 0),
                            stop=(kt == KD - 1),
                        )
                if g == 2:
                    # v_ext <- pm * (SCALE * krec) (broadcast per head)
                    kr = krec.rearrange("p (h o) -> p h o", o=1)
                    krs = work.tile([P, H], F32, tag="krs")
                    nc.vector.tensor_scalar_mul(krs, krec, SCALE)
                    nc.vector.tensor_tensor(
                        v_ext[:, lt, :, 0:DH],
                        pm.rearrange("p (h e) -> p h e", e=DH),
                        krs.rearrange("p (h o) -> p h o", o=1).to_broadcast(
                            (P, H, DH)),
                        AX.mult,
                    )
                    nc.scalar.copy(v_ext[:, lt, :, DH:SE], kr)
                else:
                    # elu(x)+1 == relu(x) + min(exp(x), 1)
                    dst = qk_sb[:, lt, g * D:(g + 1) * D]
                    if USE_STT:
                        e1 = work.tile([P, D], BF, tag="fmap_e")
                        nc.scalar.activation(e1, pm, ACTF.Exp)
                        em = work.tile([P, D], BF, tag="fmap_m")
                        nc.vector.tensor_scalar_min(em, e1, 1.0)
                        nc.vector.scalar_tensor_tensor(
                            out=dst, in0=pm, scalar=0.0, in1=em,
                            op0=AX.max, op1=AX.add,
                        )
                    else:
                        e1 = work.tile([P, D], BF, tag="fmap_e")
                        nc.scalar.activation(e1, pm, ACTF.Exp)
                        nc.vector.tensor_scalar_min(e1, e1, 1.0)
                        r1 = work.tile([P, D], BF, tag="fmap_r")
                        nc.scalar.activation(r1, pm, ACTF.Relu)
                        nc.vector.tensor_add(out=dst, in0=e1, in1=r1)
                    if g == 1:
                        ksum = work.tile([P, H], F32, tag="ksum")
                        nc.vector.reduce_sum(
                            out=ksum,
                            in_=dst.rearrange("p (h e) -> p h e", e=DH),
                            axis=mybir.AxisListType.X,
                        )
                        krec = work.tile([P, H], F32, tag="krec")
                        nc.vector.reciprocal(krec, ksum)
            nc.scalar.dma_start_transpose(qkT[:, lt], qk_sb[:, lt])

        def attn_scores(c):
            # scores for the 8 heads, parity-grouped per PSUM bank (mixing
            # PE base partitions within one bank back-to-back crashes the
            # exec unit), then inter + S-update. pb gets its own ring tiles
            # so inter does NOT wait for the masks (which read the scores
            # banks).
            ams = []
            abpb = []
            for par in range(2):  # bank holds heads {par, par+2, ...}
                ab = pab.tile([P, 512], F32, tag="ab", name="ab")
                abpb.append(ab)
                ho = par * DH
                for hh in range(4):
                    h = 2 * hh + par
                    nc.tensor.matmul(
                        ab[:, hh * P:(hh + 1) * P],
                        lhsT=qkT[ho:ho + DH, c, 4 + h // 2, :],
                        rhs=qkT[ho:ho + DH, c, h // 2, :],
                        start=True, stop=True,
                    )
                am = work.tile([P, 4, P], BF, tag="am", name="am")
                nc.vector.tensor_tensor(
                    am, ab.rearrange("p (j t) -> p j t", j=4), utri4,
                    AX.mult)
                ams.append(am)
            pbt_a = pab.tile([P, 512], F32, tag="ab", name="pbA")
            pbt_b = pab.tile([P, 512], F32, tag="ab", name="pbB")
            pba = pbt_a[:, 0:3 * PBW].rearrange("p (t e) -> p t e", e=PBW)
            pbb = pbt_b[:, 0:PBW].rearrange("p (t e) -> p t e", e=PBW)

            def pb_of(p):
                return pba[:, p, :] if p < 3 else pbb[:, 0, :]

            return ams, pba, pbb, pb_of

        def attn_pairs(c, ams, pba, pbb, pb_of):
            # Pair-contiguous inter -> intra -> S: a start=True into a bank
            # invalidates other regions' pending accumulation, so each
            # pair's group must fully finish before the next pair opens.
            for p in range(4):
                pb = pb_of(p)
                first = True
                for par in range(2):
                    if c <= par:
                        continue  # that accumulator is still empty
                    nc.tensor.matmul(pb, lhsT=qkT[:, c, p, :],
                                     rhs=s_ext2[:, par, p, :],
                                     start=first, stop=False,
                                     skip_group_check=True)
                    first = False
                for j in range(2):
                    h = 2 * p + j
                    amh = ams[h % 2][:, h // 2, :]
                    nc.tensor.matmul(
                        pb[:, j * SE:(j + 1) * SE],
                        lhsT=amh,
                        rhs=v_ext[:, c, h, :],
                        start=first, stop=(j == 1),
                        skip_group_check=True)
                # S update into this chunk's parity accumulator. EPS is
                # dropped: den is strictly positive. stop=True each chunk
                # is a hardware no-op but closes the sim accumulation group
                # so the s_ext copies may read the state.
                par = c % 2
                sps = s_unit(par, p)
                if c < NLT - 1:
                    nc.tensor.matmul(
                        sps,
                        lhsT=qk_sb[:, c,
                                   D + 2 * p * DH:D + (2 * p + 2) * DH],
                        rhs=v_ext[:, c, 2 * p:2 * p + 2, :],
                        start=(c == par), stop=True,
                        skip_group_check=True)
                    # refresh this parity's SBUF snapshot; next read is at
                    # chunk c+2, so these copies are off the critical path.
                    nc.scalar.copy(s_ext2[0:DH, par, p, 0:SE],
                                   sps[0:DH, 0:SE])
                    nc.scalar.copy(s_ext2[DH:P, par, p, SE:PBW],
                                   sps[DH:P, SE:PBW])


        def attn_epi(c, ams, pba, pbb, pb_of):
            # epilogue: strided den cols -> reciprocal -> scale out cols
            den = work.tile([P, H], F32, tag="den")
            nc.scalar.copy(
                den[:, 0:6].rearrange("p (f o) -> p f o", o=1),
                pba.rearrange("p t (j e) -> p (t j) e", e=SE)[:, :, DH:SE])
            nc.scalar.copy(
                den[:, 6:8].rearrange("p (f o) -> p f o", o=1),
                pbb.rearrange("p t (j e) -> p (t j) e", e=SE)[:, :, DH:SE])
            denr = work.tile([P, H], F32, tag="denr")
            nc.vector.reciprocal(denr, den)
            dr8 = denr.rearrange("p (j o) -> p j o", o=1)
            nc.vector.tensor_tensor(
                attn[:, c, 0:6 * DH].rearrange("p (j e) -> p j e", e=DH),
                pba.rearrange("p t (j e) -> p (t j) e", e=SE)[:, :, 0:DH],
                dr8[:, 0:6].to_broadcast((P, 6, DH)),
                AX.mult,
            )
            nc.vector.tensor_tensor(
                attn[:, c, 6 * DH:D].rearrange("p (j e) -> p j e", e=DH),
                pbb.rearrange("p t (j e) -> p (t j) e", e=SE)[:, :, 0:DH],
                dr8[:, 6:8].to_broadcast((P, 2, DH)),
                AX.mult,
            )
            nc.sync.dma_start_transpose(attnT[:, c], attn[:, c])
            if DEBUG_ATTN:
                nc.sync.dma_start(attn_dbg[:, c], attn[:, c])

        def outproj_phase(lt):
            # Matmuls + stats only; every mid-stream ACT op here is a Copy,
            # which lives in the same act table as the feature map's Exp -
            # no act-table reloads until the final ln_phase.
            pm = pmm.tile([P, D], F32, tag="mm")
            for kt in range(KD):
                nc.tensor.matmul(pm, lhsT=attnT[:, lt, kt, :],
                                 rhs=wo_b[:, kt], start=(kt == 0),
                                 stop=False)
            # residual add on the PE: pm += I^T @ h
            nc.tensor.matmul(pm, lhsT=ident_sb, rhs=h_bf[:, lt],
                             start=False, stop=True)
            nc.scalar.copy(x_all[:, lt], pm)
            stats = work.tile([P, nc.vector.BN_STATS_DIM], F32, tag="stats")
            nc.vector.bn_stats(out=stats, in_=x_all[:, lt])
            nc.vector.bn_aggr(out=mv_all[:, lt], in_=stats)

        def ln_phase(lt):
            mv = mv_all[:, lt]
            std = work.tile([P, 1], F32, tag="std")
            nc.scalar.activation(std, mv[:, 1:2], ACTF.Sqrt, bias=eps_sb,
                                 scale=1.0)
            rstd = work.tile([P, 1], F32, tag="rstd")
            nc.vector.reciprocal(rstd, std)
            nmr = work.tile([P, 1], F32, tag="nmr")
            nc.vector.tensor_scalar(out=nmr, in0=mv[:, 0:1], scalar1=-1.0,
                                    scalar2=rstd, op0=AX.mult, op1=AX.mult)
            xn = work.tile([P, D], F32 if not apply_gb else BF, tag="xn")
            nc.scalar.activation(xn, x_all[:, lt], ACTF.Identity, bias=nmr,
                                 scale=rstd)
            if apply_gb:
                xg = work.tile([P, D], BF, tag="xg")
                nc.vector.tensor_tensor(xg, xn, gamma_bc, AX.mult)
                yo = work.tile([P, D], F32, tag="yo")
                nc.vector.tensor_tensor(yo, xg, beta_bc, AX.add)
                nc.scalar.dma_start(out_d[lt * P:(lt + 1) * P, :], yo)
            else:
                nc.scalar.dma_start(out_d[lt * P:(lt + 1) * P, :], xn)

        # ---------- issue order ----------
        qkv_phase(0)
        qkv_phase(1)
        for c in range(NLT):
            st = attn_scores(c)
            if c + 2 < NLT:
                qkv_phase(c + 2)
            if c >= 1:
                outproj_phase(c - 1)
            attn_pairs(c, *st)
            attn_epi(c, *st)
        outproj_phase(NLT - 1)
        for lt in range(NLT):
            ln_phase(lt)


_NC_CACHE = {}


def _get_nc(apply_gb=True):
    key = ("nc", apply_gb)
    if key not in _NC_CACHE:
        nc = bacc.Bacc("TRN2", target_bir_lowering=False, debug=False)
        with tile.TileContext(nc) as tc:
            _build_core_kernel(nc, tc, apply_gb=apply_gb)
        nc.compile()
        _NC_CACHE[key] = nc
    return _NC_CACHE[key]


def kernel(h, W_qkv, W_o, gamma, beta, trace=False):
    global LAST_RESULT
    h = np.asarray(h, dtype=np.float32)
    W_qkv = np.asarray(W_qkv, dtype=np.float32)
    W_o = np.asarray(W_o, dtype=np.float32)
    gamma = np.asarray(gamma, dtype=np.float32)
    beta = np.asarray(beta, dtype=np.float32)

    import ml_dtypes
    bf16 = ml_dtypes.bfloat16
    fp8 = ml_dtypes.float8_e4m3 if QKV_MODE != "bf16" else ml_dtypes.bfloat16
    # Permute W_qkv columns from per-head [q|k|v] interleave to [Q|K|V]
    # blocks, partition-major fp8.
    w_perm = np.ascontiguousarray(
        W_qkv.reshape(D, H, 3, DH).transpose(0, 2, 1, 3).reshape(D, F)
        .reshape(KD, P, F).transpose(1, 0, 2)).astype(fp8)
    wo_shuf = np.ascontiguousarray(
        W_o.reshape(KD, P, D).transpose(1, 0, 2)).astype(bf16)

    apply_gb = not (np.all(gamma == 1.0) and np.all(beta == 0.0))
    nc = _get_nc(apply_gb)
    in_maps = []
    for b in range(8):
        hb = h[:, b, :]
        in_maps.append({
            "ident": np.eye(P, dtype=np.float32).astype(bf16),
            "h": np.ascontiguousarray(
                hb.reshape(NLT, P, D).transpose(1, 0, 2)).astype(bf16),
            "hT8": np.ascontiguousarray(
                hb.reshape(NLT, P, KD, P).transpose(3, 0, 2, 1)).astype(fp8),
            "W_qkv8": w_perm,
            "W_o": wo_shuf,
            "gamma": gamma,
            "beta": beta,
        })
    res = run_bass_kernel_spmd(nc, in_maps, core_ids=list(range(8)), trace=trace)
    LAST_RESULT = res
    return np.stack([res.results[b]["out"] for b in range(8)], axis=1)
